# revision 22
# baseline (speedup 1.0000x reference)
"""Trainium2 distributed kernel for ArlowVisionAttention.

Reference computation (S=4096, E=1280, H=16 heads, D=80):
    qkv = hidden @ w_qkv + b_qkv -> q,k,v per head
    q,k = RoPE(q), RoPE(k)  (interleaved rotate-half, cos/sin per (s,d))
    out_h = softmax(q_h k_h^T / sqrt(D)) v_h
    out = concat_h(out_h) @ w_proj + b_proj

Sharding: tensor-parallel over heads, 2 heads per core on 8 NeuronCores.
Each core computes its 2 heads' attention plus its partial output
projection (contraction over its 160 head-dims); the host sums the 8
partials and adds b_proj (the unshard step for a reduce-sharded output).

Per-core device program:
  - hidden^T is passed pre-transposed (and bf16-rounded) from the host;
    q^T,k^T,v^T come out of the projection directly in [dim, seq] layout
    (bf16 matmuls, fp32 PSUM accumulation; biases folded into the
    per-partition bias port of the ScalarE PSUM->SBUF copies).  v^T is
    transposed back to natural [seq, dim] bf16 layout by the DMA xbar,
    with a ones column appended for softmax denominators.
  - RoPE: rot(q) = q @ R for a constant 80x80 +-1 permutation matrix, so
    rot runs on the TensorE; cos/sin multiplies on VectorE in bf16 2x
    mode.  The 1/sqrt(D) scale is folded into w_q on the host.
  - scores are computed TRANSPOSED [st, sq] so no transposes are needed
    anywhere in the attention inner loop; exp on ScalarE over 1024-wide
    2-bank PSUM tiles (fp32 in, bf16 out; no max-subtraction needed:
    |scores| < ~3 here); the bf16 PV matmul accumulates over st in PSUM
    and the ones column of v yields the softmax denominators for free.
  - normalization happens right at the PV output: reciprocal of the
    denominator row, gpsimd partition-broadcast, one VectorE multiply
    while copying PSUM->outT (float32r).  The output projection is a
    plain two-matmul fp32r PSUM accumulation over heads + copy + DMA,
    interleaved with attention per sq-range so there is no serial tail.
"""

import numpy as np
import ml_dtypes

import concourse.bass as bass
import concourse.mybir as mybir
import concourse.tile as tile
from concourse import bacc
from concourse.bass_utils import run_bass_kernel_spmd

S = 4096
E = 1280
HEADS = 16
D = 80
N_CORES = 8
HLOC = HEADS // N_CORES  # 2 heads per core

SC = 512                 # matmul moving free dim
WC = 1024                # wide sq chunk for exp tiles (2 PSUM banks)
NWC = S // WC            # 4
NSC = S // SC            # 8
ST = 128                 # seq tile (partition dim)
NST = S // ST            # 32
KT = 128                 # contraction tile
NKT = E // KT            # 10
VW = 97                  # v block width: v(80) | zeros(16) | one @96 (32-aligned)
NG = 3 * HLOC            # 6 projection groups: qA kA qB kB vA vB

F32 = mybir.dt.float32
R32 = mybir.dt.float32r
BF16 = mybir.dt.bfloat16
NPBF16 = ml_dtypes.bfloat16

AF = mybir.ActivationFunctionType


def rot_matrix() -> np.ndarray:
    """R such that (q @ R) == rotate_half(q): out[2i]=-q[2i+1], out[2i+1]=q[2i]."""
    R = np.zeros((D, D), dtype=np.float32)
    for i in range(D // 2):
        R[2 * i + 1, 2 * i] = -1.0
        R[2 * i, 2 * i + 1] = 1.0
    return R


def build_program():
    nc = bacc.Bacc(None, target_bir_lowering=False)

    # packed projection weights: 6 groups [qA kA qB kB vA vB] of D cols each
    hT = nc.declare_dram_parameter("hT", [E, S], BF16, False)
    wt = nc.declare_dram_parameter("wt", [E, NG * D], BF16, False)
    bt = nc.declare_dram_parameter("bt", [D, NG], F32, False)
    cosT = nc.declare_dram_parameter("cosT", [D, S], BF16, False)
    sinT = nc.declare_dram_parameter("sinT", [D, S], BF16, False)
    wp = nc.declare_dram_parameter("wp", [2 * D, E], BF16, False)
    rmat = nc.declare_dram_parameter("rmat", [D, D], BF16, False)
    out = nc.declare_dram_parameter("out", [S, E], F32, True)

    with tile.TileContext(nc) as tc:
        with tc.tile_pool(name="const", bufs=1) as cpool:
            # ---- persistent tensors ----
            wt_sb = cpool.tile([KT, NKT * NG * D], BF16)  # block k: wt rows k*128..
            bt_sb = cpool.tile([D, NG], F32)
            wp_sb = cpool.tile([D, 2 * E], BF16)           # head h at cols h*E..
            r_sb = cpool.tile([D, D], BF16)
            ident = cpool.tile([D, D], BF16)
            q_sb = cpool.tile([D, 2 * S], BF16)           # head h at cols h*S..
            k_sb = cpool.tile([D, 2 * S], BF16)
            v_sb = cpool.tile([ST, 2 * NST * VW], BF16)   # [st 128, (head,stile)*97]
            outT = cpool.tile([D, 2 * S], BF16)

            for k in range(NKT):
                nc.gpsimd.dma_start(
                    wt_sb[:, k * NG * D:(k + 1) * NG * D],
                    wt[k * KT:(k + 1) * KT, :],
                )
            nc.gpsimd.dma_start(bt_sb[:], bt[:])
            for h in range(HLOC):
                nc.gpsimd.dma_start(
                    wp_sb[:, h * E:(h + 1) * E], wp[h * D:(h + 1) * D, :]
                )
            nc.gpsimd.dma_start(r_sb[:], rmat[:])
            from concourse.masks import make_identity
            make_identity(nc, ident[:])
            # pad columns (zeros) and ones column of v blocks, via an f32
            # const tile broadcast-copied into the bf16 tensor
            pad_src = cpool.tile([ST, VW - D], F32)
            nc.vector.memset(pad_src[:, 0:VW - D - 1], 0.0)
            nc.vector.memset(pad_src[:, VW - D - 1:VW - D], 1.0)
            nc.vector.tensor_copy(
                v_sb.rearrange("p (b c) -> p b c", c=VW)[:, :, D:VW],
                pad_src[:].unsqueeze(1).to_broadcast([ST, 2 * NST, VW - D]),
            )

            # ---- phase 1: projections + RoPE + v transpose ----
            # group-outer, k-inner: each group's PSUM epilogue (ScalarE copy
            # with bias, RoPE rot matmul, v DMA-transposes) overlaps the
            # next group's accumulation matmuls.
            with (
                tc.tile_pool(name="p1", bufs=1) as p1pool,
                tc.tile_pool(name="ps1", bufs=1, space="PSUM") as ps1,
            ):
                for c in range(NSC):
                    htks = []
                    for k in range(NKT):
                        htk = p1pool.tile([KT, SC], BF16, tag="htk", bufs=13,
                                          name=f"htk{k}")
                        nc.sync.dma_start(
                            htk[:], hT[k * KT:(k + 1) * KT, c * SC:(c + 1) * SC]
                        )
                        htks.append(htk)
                    cos_t = p1pool.tile([D, SC], BF16, tag="cos", bufs=2)
                    sin_t = p1pool.tile([D, SC], BF16, tag="sin", bufs=2)
                    nc.gpsimd.dma_start(cos_t[:], cosT[:, c * SC:(c + 1) * SC])
                    nc.gpsimd.dma_start(sin_t[:], sinT[:, c * SC:(c + 1) * SC])
                    for g in range(NG):
                        acc = ps1.tile([D, SC], F32, tag="ps", bufs=6,
                                       name=f"acc{g}")
                        for k in range(NKT):
                            nc.tensor.matmul(
                                acc[:],
                                wt_sb[:, (k * NG + g) * D:(k * NG + g + 1) * D],
                                htks[k][:],
                                start=(k == 0),
                                stop=(k == NKT - 1),
                            )
                        if g < 4:
                            # q or k head: copy out with bias, then RoPE
                            dest = q_sb if g % 2 == 0 else k_sb
                            h = g // 2
                            chunk = dest[:, h * S + c * SC:h * S + (c + 1) * SC]
                            nc.scalar.activation(
                                chunk, acc[:], AF.Identity, bias=bt_sb[:, g:g + 1]
                            )
                            rp = ps1.tile([D, SC], F32, tag="ps", bufs=6,
                                          name="rot")
                            nc.tensor.matmul(
                                rp[:], r_sb[:], chunk, start=True, stop=True
                            )
                            tmp = p1pool.tile([D, SC], BF16, tag="rtmp", bufs=2)
                            nc.vector.tensor_mul(tmp[:], sin_t[:], rp[:])
                            nc.vector.tensor_mul(chunk, chunk, cos_t[:])
                            nc.vector.tensor_add(chunk, chunk, tmp[:])
                        else:
                            # v head: copy out with bias (bf16), then DMA-xbar
                            # transpose to natural layout
                            h = g - 4
                            vt = p1pool.tile([D, SC], BF16, tag="vt", bufs=2)
                            nc.scalar.activation(
                                vt[:], acc[:], AF.Identity,
                                bias=bt_sb[:, g:g + 1],
                            )
                            for t in range(SC // ST):
                                j = h * NST + c * (SC // ST) + t
                                trp = ps1.tile([ST, D], BF16, tag="ps", bufs=6,
                                               name="trp")
                                nc.tensor.transpose(
                                    trp[:], vt[:, t * ST:(t + 1) * ST], ident[:]
                                )
                                nc.vector.tensor_copy(
                                    v_sb[:, j * VW:j * VW + D], trp[:]
                                )

            # ---- phase 2+3: attention with interleaved output projection ----
            with (
                tc.tile_pool(name="p2", bufs=1) as p2pool,
                tc.tile_pool(name="ps2", bufs=1, space="PSUM") as ps2,
                tc.tile_pool(name="dram2", bufs=1, space="DRAM") as drampool,
            ):
                ECH = [(0, 512), (512, 512), (1024, 256)]

                def emit_proj(cp, half=None):
                    j0 = cp * (WC // ST)
                    jn = WC // ST
                    if half == 0:
                        rng = range(j0, j0 + jn // 2)
                    elif half == 1:
                        rng = range(j0 + jn // 2, j0 + jn)
                    else:
                        rng = range(j0, j0 + jn)
                    for j in rng:
                        for (e0, ew) in ECH:
                            fp = ps2.tile([ST, SC], F32, tag="fp", bufs=2,
                                          name="fp")
                            nc.tensor.matmul(
                                fp[:, :ew],
                                outT[:, 0 * S + j * ST:0 * S + (j + 1) * ST],
                                wp_sb[:, 0 * E + e0:0 * E + e0 + ew],
                                start=True, stop=False,
                            )
                            nc.tensor.matmul(
                                fp[:, :ew],
                                outT[:, 1 * S + j * ST:1 * S + (j + 1) * ST],
                                wp_sb[:, 1 * E + e0:1 * E + e0 + ew],
                                start=False, stop=True,
                            )
                            t0 = p2pool.tile([ST, SC], F32, tag="t0", bufs=3,
                                             name="t0")
                            nc.vector.tensor_copy(t0[:, :ew], fp[:, :ew])
                            nc.sync.dma_start(
                                out[j * ST:(j + 1) * ST, e0:e0 + ew], t0[:, :ew]
                            )

                for c in range(NWC):
                    for h in range(HLOC):
                        q0 = h * S + c * WC
                        pv0 = ps2.tile([VW, SC], F32, tag="pv", bufs=2, name="pv0")
                        pv1 = ps2.tile([VW, SC], F32, tag="pv", bufs=2, name="pv1")
                        for st in range(NST):
                            sp = ps2.tile([ST, WC], F32, tag="sc", bufs=2)
                            kblk = k_sb[:, h * S + st * ST:h * S + (st + 1) * ST]
                            nc.tensor.matmul(
                                sp[:, 0:SC], kblk, q_sb[:, q0:q0 + SC],
                                start=True, stop=True,
                            )
                            nc.tensor.matmul(
                                sp[:, SC:WC], kblk, q_sb[:, q0 + SC:q0 + WC],
                                start=True, stop=True,
                            )
                            ex = p2pool.tile([ST, WC], BF16, tag="exp", bufs=3)
                            nc.scalar.activation(ex[:], sp[:], AF.Exp)
                            vblk = v_sb[:, (h * NST + st) * VW:(h * NST + st + 1) * VW]
                            nc.tensor.matmul(
                                pv0[:], vblk, ex[:, 0:SC],
                                start=(st == 0), stop=(st == NST - 1),
                            )
                            nc.tensor.matmul(
                                pv1[:], vblk, ex[:, SC:WC],
                                start=(st == 0), stop=(st == NST - 1),
                            )
                        # free the PV PSUM slots fast: copy to SBUF, then
                        # normalize off the critical path.  The reciprocal is
                        # made lane-parallel by bouncing the denominator row
                        # through DRAM into a [128, 8] layout (a 1-partition
                        # reciprocal would serialize 8.5us on the DVE), and
                        # the partition broadcast is a DRAM-source
                        # broadcast-AP DMA read.
                        pvs = p2pool.tile([VW, WC], F32, tag="pvs", bufs=3)
                        nc.vector.tensor_copy(pvs[:, 0:SC], pv0[:])
                        nc.vector.tensor_copy(pvs[:, SC:WC], pv1[:])
                        dend = drampool.tile([WC], F32, tag="dend", bufs=2,
                                             name="dend")
                        nc.sync.dma_start(
                            dend[:].rearrange("(o f) -> o f", o=1),
                            pvs[VW - 1:VW, :],
                        )
                        d128 = p2pool.tile([ST, WC // ST], F32, tag="d128",
                                           bufs=2)
                        nc.sync.dma_start(
                            d128[:], dend[:].rearrange("(j p) -> p j", p=ST)
                        )
                        r128 = p2pool.tile([ST, WC // ST], F32, tag="r128",
                                           bufs=2)
                        nc.vector.reciprocal(r128[:], d128[:])
                        rcd = drampool.tile([WC], F32, tag="rcd", bufs=2,
                                            name="rcd")
                        nc.sync.dma_start(
                            rcd[:].rearrange("(j p) -> p j", p=ST), r128[:]
                        )
                        bc = p2pool.tile([D, WC], F32, tag="bc", bufs=2)
                        nc.sync.dma_start(
                            bc[:],
                            bass.AP(rcd.tensor, rcd[:].offset,
                                    [[0, D]] + list(rcd[:].ap)),
                        )
                        nc.vector.tensor_mul(
                            outT[:, q0:q0 + WC], pvs[0:D, :], bc[:]
                        )
                        # output projection for the previous wide chunk (both
                        # heads done there); keeps the PE fed across the
                        # normalize chain at every chunk boundary
                        if c > 0:
                            emit_proj(c - 1, half=h)
                if True:
                    emit_proj(NWC - 1)

    nc.compile()
    return nc


def core_inputs(inputs: dict, c: int) -> dict:
    """Build the per-core input map (host-side shard + repack)."""
    hs = np.asarray(inputs["hidden_states"], dtype=np.float32)
    cos = np.asarray(inputs["cos"], dtype=np.float32)
    sin = np.asarray(inputs["sin"], dtype=np.float32)
    w_qkv = np.asarray(inputs["w_qkv"], dtype=np.float32)
    b_qkv = np.asarray(inputs["b_qkv"], dtype=np.float32)
    w_proj = np.asarray(inputs["w_proj"], dtype=np.float32)

    scale = np.float32(D ** -0.5)
    hA, hB = HLOC * c, HLOC * c + 1

    def wcol(kind, h):  # kind 0=q 1=k 2=v
        return w_qkv[:, kind * E + h * D:kind * E + (h + 1) * D]

    def bcol(kind, h):
        return b_qkv[kind * E + h * D:kind * E + (h + 1) * D]

    # groups: qA kA qB kB vA vB (q pre-scaled by 1/sqrt(D))
    wt = np.concatenate(
        [wcol(0, hA) * scale, wcol(1, hA), wcol(0, hB) * scale, wcol(1, hB),
         wcol(2, hA), wcol(2, hB)], axis=1)
    bt = np.stack(
        [bcol(0, hA) * scale, bcol(1, hA), bcol(0, hB) * scale, bcol(1, hB),
         bcol(2, hA), bcol(2, hB)], axis=1)
    wp = np.ascontiguousarray(w_proj[hA * D:(hB + 1) * D, :])

    return {
        "hT": np.ascontiguousarray(hs.T).astype(NPBF16),
        "wt": np.ascontiguousarray(wt).astype(NPBF16),
        "bt": np.ascontiguousarray(bt),
        "cosT": np.ascontiguousarray(cos.T).astype(NPBF16),
        "sinT": np.ascontiguousarray(sin.T).astype(NPBF16),
        "wp": wp.astype(NPBF16),
        "rmat": rot_matrix().astype(NPBF16),
    }


def core_partial_ref(inputs: dict, c: int) -> np.ndarray:
    """Numpy reference for one core's partial output (for debugging)."""
    ci = core_inputs(inputs, c)
    h = ci["hT"].T.astype(np.float32)
    R = ci["rmat"].astype(np.float32)
    cos = ci["cosT"].T.astype(np.float32)
    sin = ci["sinT"].T.astype(np.float32)
    wt = ci["wt"].astype(np.float32)
    partial = np.zeros((S, E), dtype=np.float32)
    for hh in range(HLOC):
        q = h @ wt[:, (2 * hh) * D:(2 * hh + 1) * D] + ci["bt"][:, 2 * hh]
        k = h @ wt[:, (2 * hh + 1) * D:(2 * hh + 2) * D] + ci["bt"][:, 2 * hh + 1]
        v = h @ wt[:, (4 + hh) * D:(4 + hh + 1) * D] + ci["bt"][:, 4 + hh]
        q = q * cos + (q @ R) * sin
        k = k * cos + (k @ R) * sin
        s = q @ k.T
        e = np.exp(s)
        a = e / e.sum(axis=-1, keepdims=True)
        o = a @ v
        partial += o @ ci["wp"][hh * D:(hh + 1) * D, :].astype(np.float32)
    return partial


_NC_CACHE = {}


def _get_program():
    if "nc" not in _NC_CACHE:
        _NC_CACHE["nc"] = build_program()
    return _NC_CACHE["nc"]


def kernel(**inputs) -> np.ndarray:
    nc = _get_program()
    in_maps = [core_inputs(inputs, c) for c in range(N_CORES)]
    res = run_bass_kernel_spmd(nc, in_maps, core_ids=list(range(N_CORES)))
    b_proj = np.asarray(inputs["b_proj"], dtype=np.float32)
    total = np.zeros((S, E), dtype=np.float32)
    for c in range(N_CORES):
        total += res.results[c]["out"]
    return total + b_proj[None, :]


if __name__ == "__main__":
    import reference

    inputs = {k: np.asarray(v) for k, v in reference.setup_inputs().items()}
    expected = np.asarray(reference.reference(**inputs))
    actual = kernel(**inputs)
    rms_rel = np.linalg.norm(actual - expected) / np.linalg.norm(expected)
    print(f"rms rel err: {rms_rel:.3e}")
